# revision 19
# baseline (speedup 1.0000x reference)
"""Trainium2 Bass kernel: transformer decoder layer (self-attn + cross-attn +
top-2-of-8 MoE), expert-parallel dense MoE across 8 NeuronCores.

Per-core SPMD program (core c; batch b=c//2, half h=c%2, expert e=c):
  - attention phases computed data-parallel: 512 query tokens per core, in a
    transposed [d, token] activation layout (keeps every matmul K-major).
  - x after cross-attn is AllGathered (bf16); each core runs its one expert's
    FFN densely over all 4096 tokens, scaled by the routing gate column, and
    a ReduceScatter sums expert contributions back to the token's owner core.
  - top-2 routing computed on the token's owner core in fp32.
Host preprocessing: transposes activations, pre-scales Wq by 1/sqrt(dh),
permutes each batch so the core's own query tokens are always columns 0:512.
"""

import sys

sys.path.insert(0, "/opt/trn_rl_repo")
import numpy as np

N_CORES = 8
D = 1024
DFF = 4096
NH = 16
DH = 64
TOK = 512
KV1 = 1024
KV2 = 512
P = 128
KT = D // P  # 8 contraction tiles of 128
EPS = 1e-6

_cache = {}


def _build():
    import concourse.mybir as mybir
    import concourse.tile as tile
    from concourse import bacc
    from concourse.alu_op_type import AluOpType

    f32 = mybir.dt.float32
    f16 = mybir.dt.float16
    f32r = mybir.dt.float32r
    bf16 = mybir.dt.bfloat16
    i32 = mybir.dt.int32
    u32 = mybir.dt.uint32
    ActF = mybir.ActivationFunctionType
    AX = mybir.AxisListType.X

    nc = bacc.Bacc("TRN2", target_bir_lowering=False, debug=False,
                   num_devices=N_CORES)

    decT = nc.dram_tensor("decT", [D, KV1], f32, kind="ExternalInput")
    encT = nc.dram_tensor("encT", [D, KV2], f32, kind="ExternalInput")
    wnames = ["wq1", "wk1", "wv1", "wo1", "wq2", "wk2", "wv2", "wo2"]
    WD = {n: nc.dram_tensor(n, [D, D], f32, kind="ExternalInput") for n in wnames}
    gdram = nc.dram_tensor("gv", [P, KT], f32, kind="ExternalInput")
    wgdram = nc.dram_tensor("wg", [P, KT * 8], f32, kind="ExternalInput")
    identd = nc.dram_tensor("ident", [P, P], f32, kind="ExternalInput")
    onesd = nc.dram_tensor("ones1", [P, NH], f32, kind="ExternalInput")
    w1d = nc.dram_tensor("w1e", [D, DFF], bf16, kind="ExternalInput")
    w2d = nc.dram_tensor("w2e", [DFF, D], bf16, kind="ExternalInput")

    xoutd = nc.dram_tensor("xoutT", [D, TOK], f32, kind="ExternalOutput")
    woutd = nc.dram_tensor("wout", [TOK, 2], f32, kind="ExternalOutput")
    ioutd = nc.dram_tensor("iout", [TOK, 2], i32, kind="ExternalOutput")

    with tile.TileContext(nc) as tc:
        with (
            tc.tile_pool(name="const", bufs=1) as constp,
            tc.tile_pool(name="psum", bufs=1, space="PSUM") as psp,
            tc.tile_pool(name="dram", bufs=1, space="DRAM") as dramp,
            tc.tile_pool(name="rows", bufs=3) as rowp,
        ):
            g_sb = constp.tile([P, KT], f32)
            nc.sync.dma_start(g_sb[:], gdram[:])
            wg_sb = constp.tile([P, KT * 8], f32)
            nc.sync.dma_start(wg_sb[:], wgdram[:])
            ident = constp.tile([P, P], f32)
            nc.sync.dma_start(ident[:], identd[:])
            ones_r = constp.tile([P, NH], f32r)
            nc.sync.dma_start(ones_r[:], onesd[:].bitcast(f32r))
            eps_row = constp.tile([1, 1], f32)
            nc.vector.memset(eps_row[:], EPS)

            # internal DRAM
            ag_in = dramp.tile([D, TOK], bf16)
            ag_out = dramp.tile([N_CORES * D, TOK], bf16, addr_space="Shared")
            a2a_in = dramp.tile([N_CORES, TOK], f32)
            a2a_out = dramp.tile([N_CORES, TOK], f32)
            rs_in = dramp.tile([N_CORES * D, TOK], f16)
            rs_out = dramp.tile([D, TOK], f16)
            x2dram = dramp.tile([D, TOK], f32)

            def ps_mm(n=TOK):
                return psp.tile([P, n], f32, tag="ps_mm", name="ps_mm", bufs=4)

            def w_stream(wt_dram, wtag, wpool, M=D, dt=f32r):
                tiles = []
                for k in range(KT):
                    t = wpool.tile([P, M], dt, tag=wtag, name=wtag)
                    nc.sync.dma_start(t[:], wt_dram[k * P:(k + 1) * P, :].bitcast(dt))
                    tiles.append(t)
                return tiles

            def proj(w_tiles, rhs_tiles, rhs_sl, evict, n_chunks=1):
                # out[m][nc_] = sum_k w_tiles[k][:,m*P:+P].T @ rhs_tiles[k][:, sl]
                for nci in range(n_chunks):
                    for m in range(KT):
                        ps = ps_mm()
                        for k in range(KT):
                            rhs = rhs_tiles[k][:] if rhs_sl is None else rhs_tiles[k][:, rhs_sl(nci)]
                            nc.tensor.matmul(
                                ps[:], w_tiles[k][:, m * P:(m + 1) * P], rhs,
                                start=(k == 0), stop=(k == KT - 1))
                        evict(nci, m, ps)

            def rmsnorm_apply(src_tiles, out_dtype, out_tag, out_pool,
                              sq_pool=None, sq_tag="sq", sq_bufs=8):
                # rmsnorm over the partition (d) axis of 8x[128, TOK] tiles
                if sq_pool is None:
                    sq_pool = out_pool
                sq = []
                for m in range(KT):
                    s = sq_pool.tile([P, TOK], f32r, tag=sq_tag, name="sq", bufs=sq_bufs)
                    nc.vector.tensor_mul(s[:], src_tiles[m][:], src_tiles[m][:])
                    sq.append(s)
                ps_ss = psp.tile([1, TOK], f32, tag="ps_small", name="ps_ss", bufs=2)
                for m in range(KT):
                    nc.tensor.matmul(ps_ss[:], ones_r[:, 0:1], sq[m][:],
                                     start=(m == 0), stop=(m == KT - 1))
                sqms = rowp.tile([1, TOK], f32, tag="sqms", name="sqms", bufs=1)
                nc.scalar.activation(sqms[:], ps_ss[:], ActF.Sqrt,
                                     scale=1.0 / D, bias=eps_row[:])
                rstd = rowp.tile([1, TOK], f32, tag="rstd", name="rstd", bufs=1)
                nc.vector.reciprocal(rstd[:], sqms[:])
                rbc = out_pool.tile([P, TOK], f32, tag="rbc", name="rbc", bufs=1)
                nc.gpsimd.partition_broadcast(rbc[:], rstd[:])
                outs = []
                for m in range(KT):
                    o = out_pool.tile([P, TOK], out_dtype, tag=out_tag, name=out_tag, bufs=8)
                    nc.vector.scalar_tensor_tensor(
                        o[:], src_tiles[m][:], g_sb[:, m:m + 1], rbc[:],
                        AluOpType.mult, AluOpType.mult)
                    outs.append(o)
                return outs

            def load_kv_tiles(kvT_dram, kv_len, apool):
                kv_sb = []
                for k in range(KT):
                    t = apool.tile([P, kv_len], f32r, tag="kvsrc",
                                   name="kvsrc", bufs=8)
                    nc.sync.dma_start(
                        t[:], kvT_dram[k * P:(k + 1) * P, :].bitcast(f32r))
                    kv_sb.append(t)
                return kv_sb

            def attention(kv_sb, kv_len, wq, wk, wv, wo,
                          res_fn, qsrc_fn, apool, wpool, out_pool,
                          out_dtype, out_tag):
                # transposed-layout attention for TOK query tokens.
                # qsrc_fn(k) -> [128, TOK] f32r rhs tile for the Q projection
                # res_fn(m) -> [128, TOK] f32-readable residual tile
                KVT = kv_len // P
                # Q^T = wq.T @ qsrc   [d, TOK]
                wq_t = w_stream(wq, "wproj", wpool)
                qt = []
                for m in range(KT):
                    ps = ps_mm()
                    for k in range(KT):
                        nc.tensor.matmul(ps[:], wq_t[k][:, m * P:(m + 1) * P],
                                         qsrc_fn(k), start=(k == 0),
                                         stop=(k == KT - 1))
                    t = apool.tile([P, TOK], f32r, tag="qt", name="qt", bufs=8)
                    nc.vector.tensor_copy(t[:], ps[:])
                    qt.append(t)

                # V natural [kv_tok, dv] with ones column per head (width 65)
                wv_t = w_stream(wv, "wproj", wpool)
                v_sb = []
                for t_i in range(KVT):
                    vt = apool.tile([P, NH * (DH + 1)], f32r, tag="v",
                                    name="vt", bufs=8)
                    v3 = vt[:].rearrange("p (h x) -> p h x", x=DH + 1)
                    nc.sync.dma_start(v3[:, :, DH:DH + 1],
                                      onesd[:].bitcast(f32r))
                    for nci in range(2):
                        ps = ps_mm()
                        for k in range(KT):
                            nc.tensor.matmul(
                                ps[:], kv_sb[k][:, t_i * P:(t_i + 1) * P],
                                wv_t[k][:, nci * TOK:(nci + 1) * TOK],
                                start=(k == 0), stop=(k == KT - 1))
                        nc.vector.tensor_copy(
                            v3[:, nci * 8:(nci + 1) * 8, 0:DH], ps[:])
                    v_sb.append(vt)

                # K^T tile i streamed; feeds heads 2i and 2i+1
                wk_t = w_stream(wk, "wproj", wpool)
                at_sb = [apool.tile([P, TOK], f32r, tag="at", name="at", bufs=8)
                         for _ in range(KT)]
                for i in range(KT):
                    kt_i = apool.tile([P, kv_len], f32r, tag="kt", name="ktt",
                                      bufs=2)
                    for nci in range(kv_len // TOK):
                        ps = ps_mm()
                        for k in range(KT):
                            nc.tensor.matmul(
                                ps[:], wk_t[k][:, i * P:(i + 1) * P],
                                kv_sb[k][:, nci * TOK:(nci + 1) * TOK],
                                start=(k == 0), stop=(k == KT - 1))
                        nc.vector.tensor_copy(
                            kt_i[:, nci * TOK:(nci + 1) * TOK], ps[:])
                    for hh in (2 * i, 2 * i + 1):
                        po = DH * (hh % 2)
                        s_tiles = []
                        for t_i in range(KVT):
                            ps = ps_mm()
                            nc.tensor.matmul(
                                ps[:], kt_i[po:po + DH, t_i * P:(t_i + 1) * P],
                                qt[i][po:po + DH, :], start=True, stop=True)
                            st = apool.tile([P, TOK], f32r, tag="s", name="st",
                                            bufs=10)
                            nc.scalar.activation(st[:], ps[:], ActF.Exp)
                            s_tiles.append(st)
                        ps_av = psp.tile([DH + 1, TOK], f32, tag="ps_av",
                                         name="ps_av", bufs=2)
                        for t_i in range(KVT):
                            v3 = v_sb[t_i][:].rearrange("p (h x) -> p h x",
                                                        x=DH + 1)
                            nc.tensor.matmul(ps_av[:], v3[:, hh, :],
                                             s_tiles[t_i][:], start=(t_i == 0),
                                             stop=(t_i == KVT - 1))
                        rcp = rowp.tile([1, TOK], f32, tag="rcp", name="rcp",
                                        bufs=2)
                        nc.vector.reciprocal(rcp[:], ps_av[DH:DH + 1, :])
                        rb = apool.tile([DH, TOK], f32, tag="avbc", name="rb",
                                        bufs=2)
                        nc.gpsimd.partition_broadcast(rb[:], rcp[:])
                        nc.vector.tensor_mul(at_sb[i][po:po + DH, :],
                                             ps_av[0:DH, :], rb[:])

                # O-proj + residual (pre tiles reuse the qt slots)
                wo_t = w_stream(wo, "wproj", wpool)
                pre = []
                for m in range(KT):
                    ps = ps_mm()
                    for k in range(KT):
                        nc.tensor.matmul(ps[:], wo_t[k][:, m * P:(m + 1) * P],
                                         at_sb[k][:], start=(k == 0),
                                         stop=(k == KT - 1))
                    t = apool.tile([P, TOK], f32, tag="qt", name="pre", bufs=8)
                    nc.vector.tensor_add(t[:], ps[:], res_fn(m))
                    pre.append(t)
                return pre

            # ---------------- phase A: self attention ----------------
            with (
                tc.tile_pool(name="attn", bufs=1) as apool,
                tc.tile_pool(name="wts", bufs=8) as wpool,
                tc.tile_pool(name="act1", bufs=1) as x1pool,
            ):
                kv1 = load_kv_tiles(decT, KV1, apool)
                pre1 = attention(
                    kv1, KV1, WD["wq1"], WD["wk1"], WD["wv1"], WD["wo1"],
                    res_fn=lambda m: kv1[m][:, 0:TOK].bitcast(f32),
                    qsrc_fn=lambda k: kv1[k][:, 0:TOK],
                    apool=apool, wpool=wpool, out_pool=x1pool,
                    out_dtype=f32r, out_tag="x12")
                x1 = rmsnorm_apply(pre1, f32r, "x12", x1pool,
                                   sq_pool=apool, sq_tag="s", sq_bufs=10)

                # ------------- phase B: cross attention -------------
                kv2 = load_kv_tiles(encT, KV2, apool)
                pre2 = attention(
                    kv2, KV2, WD["wq2"], WD["wk2"], WD["wv2"], WD["wo2"],
                    res_fn=lambda m: x1[m][:].bitcast(f32),
                    qsrc_fn=lambda k: x1[k][:],
                    apool=apool, wpool=wpool, out_pool=x1pool,
                    out_dtype=f32, out_tag="x12")
                x2 = rmsnorm_apply(pre2, f32, "x12", x1pool,
                                   sq_pool=apool, sq_tag="s", sq_bufs=10)

                # export x2: f32 -> private dram; bf16 -> allgather
                for m in range(KT):
                    nc.sync.dma_start(x2dram[m * P:(m + 1) * P, :], x2[m][:])
                    xb = x1pool.tile([P, TOK], bf16, tag="xb", name="xb", bufs=1)
                    nc.vector.tensor_copy(xb[:], x2[m][:])
                    nc.sync.dma_start(ag_in[m * P:(m + 1) * P, :], xb[:])
                nc.gpsimd.collective_compute(
                    "AllGather", mybir.AluOpType.bypass,
                    replica_groups=[list(range(N_CORES))],
                    ins=[ag_in[:].opt()], outs=[ag_out[:].opt()])

                # ------------- router (own 512 tokens, fp32) -------------
                wgv = wg_sb[:].rearrange("p (k e) -> p k e", e=8)
                ct_sb = x1pool.tile([8, TOK], f32, tag="ct", bufs=1)
                for t_i in range(TOK // P):
                    ps_l = psp.tile([P, 8], f32, tag="ps_small", name="ps_l", bufs=2)
                    for k in range(KT):
                        nc.tensor.matmul(
                            ps_l[:], x2[k][:, t_i * P:(t_i + 1) * P], wgv[:, k, :],
                            start=(k == 0), stop=(k == KT - 1))
                    negm = rowp.tile([P, 1], f32, tag="negm", name="negm", bufs=2)
                    nc.vector.reduce_max(negm[:], ps_l[:], axis=AX, negate=True)
                    eb = rowp.tile([P, 8], f32, tag="eb", name="eb", bufs=2)
                    nc.scalar.activation(eb[:], ps_l[:], ActF.Exp, bias=negm[:])
                    vals = rowp.tile([P, 8], f32, tag="vals", name="vals", bufs=2)
                    idxu = rowp.tile([P, 8], u32, tag="idxu", name="idxu", bufs=2)
                    nc.vector.max_with_indices(vals[:], idxu[:], eb[:])
                    s12 = rowp.tile([P, 1], f32, tag="s12", name="s12", bufs=2)
                    nc.vector.tensor_add(s12[:], vals[:, 0:1], vals[:, 1:2])
                    rr = rowp.tile([P, 1], f32, tag="rr", name="rr", bufs=2)
                    nc.vector.reciprocal(rr[:], s12[:])
                    wo_sb = rowp.tile([P, 2], f32, tag="wo_sb", name="wo_sb", bufs=2)
                    nc.vector.tensor_scalar_mul(wo_sb[:], vals[:, 0:2], rr[:])
                    nc.sync.dma_start(woutd[t_i * P:(t_i + 1) * P, :], wo_sb[:])
                    io_sb = rowp.tile([P, 2], i32, tag="io_sb", name="io_sb", bufs=2)
                    nc.vector.tensor_copy(io_sb[:], idxu[:, 0:2])
                    nc.sync.dma_start(ioutd[t_i * P:(t_i + 1) * P, :], io_sb[:])
                    mask = rowp.tile([P, 8], f32, tag="mask", name="mask", bufs=2)
                    nc.vector.tensor_scalar(mask[:], eb[:], vals[:, 1:2], None,
                                            AluOpType.is_ge)
                    comb = rowp.tile([P, 8], f32, tag="comb", name="comb", bufs=2)
                    nc.vector.scalar_tensor_tensor(
                        comb[:], eb[:], rr[:], mask[:],
                        AluOpType.mult, AluOpType.mult)
                    ps_t = psp.tile([8, P], f32, tag="ps_small", name="ps_t", bufs=2)
                    nc.tensor.transpose(ps_t[:], comb[:], ident[:])
                    nc.vector.tensor_copy(ct_sb[:, t_i * P:(t_i + 1) * P], ps_t[:])
                nc.sync.dma_start(a2a_in[:], ct_sb[:])
                nc.gpsimd.collective_compute(
                    "AllToAll", mybir.AluOpType.bypass,
                    replica_groups=[list(range(N_CORES))],
                    ins=[a2a_in[:].opt()], outs=[a2a_out[:].opt()])

            # ---------------- phase C: dense expert FFN ----------------
            with (
                tc.tile_pool(name="moew", bufs=1) as mwp,
                tc.tile_pool(name="moea", bufs=1) as map_,
                tc.tile_pool(name="xg", bufs=16) as xgp,
                tc.tile_pool(name="ysb", bufs=3) as yp,
            ):
                w1_sb = []
                for k in range(KT):
                    t = mwp.tile([P, DFF], bf16, tag="w1", name="w1t", bufs=8)
                    nc.sync.dma_start(t[:], w1d[k * P:(k + 1) * P, :])
                    w1_sb.append(t)
                w2_sb = []
                for k in range(DFF // P):
                    t = mwp.tile([P, D], bf16, tag="w2", name="w2t", bufs=32)
                    nc.sync.dma_start(t[:], w2d[k * P:(k + 1) * P, :])
                    w2_sb.append(t)

                for r in range(N_CORES):
                    xg = []
                    for k in range(KT):
                        t = xgp.tile([P, TOK], bf16, tag="xg", name="xgt")
                        nc.sync.dma_start(
                            t[:], ag_out[r * D + k * P: r * D + (k + 1) * P, :])
                        xg.append(t)
                    cvec = rowp.tile([1, TOK], f32, tag="cvec", name="cvec", bufs=1)
                    nc.sync.dma_start(cvec[:], a2a_out[r:r + 1, :])
                    cbc = yp.tile([P, TOK], f32, tag="cbc", name="cbc", bufs=2)
                    nc.gpsimd.partition_broadcast(cbc[:], cvec[:])

                    h_sb = []
                    for mf in range(DFF // P):
                        ps = ps_mm()
                        for k in range(KT):
                            nc.tensor.matmul(
                                ps[:], w1_sb[k][:, mf * P:(mf + 1) * P], xg[k][:],
                                start=(k == 0), stop=(k == KT - 1))
                        ht = map_.tile([P, TOK], bf16, tag="h", name="ht", bufs=32)
                        nc.scalar.activation(ht[:], ps[:], ActF.Relu)
                        h_sb.append(ht)

                    for md in range(KT):
                        ps = ps_mm()
                        for kk in range(DFF // P):
                            nc.tensor.matmul(
                                ps[:], w2_sb[kk][:, md * P:(md + 1) * P], h_sb[kk][:],
                                start=(kk == 0), stop=(kk == DFF // P - 1))
                        yt = yp.tile([P, TOK], f16, tag="y", name="yt")
                        nc.vector.tensor_mul(yt[:], ps[:], cbc[:])
                        nc.sync.dma_start(
                            rs_in[r * D + md * P: r * D + (md + 1) * P, :], yt[:])

                nc.gpsimd.collective_compute(
                    "ReduceScatter", mybir.AluOpType.add,
                    replica_groups=[list(range(N_CORES))],
                    ins=[rs_in[:].opt()], outs=[rs_out[:].opt()])

            # ---------------- phase D: final residual + norm ----------------
            with tc.tile_pool(name="fin", bufs=1) as fp:
                pre = []
                for m in range(KT):
                    yt = fp.tile([P, TOK], f16, tag="yl", name="ylt", bufs=3)
                    nc.sync.dma_start(yt[:], rs_out[m * P:(m + 1) * P, :])
                    xt = fp.tile([P, TOK], f32, tag="xl", name="xlt", bufs=3)
                    nc.sync.dma_start(xt[:], x2dram[m * P:(m + 1) * P, :])
                    t = fp.tile([P, TOK], f32, tag="fpre", name="fpre", bufs=8)
                    nc.vector.tensor_add(t[:], yt[:], xt[:])
                    pre.append(t)
                xout = rmsnorm_apply(pre, f32, "xout", fp)
                for m in range(KT):
                    nc.sync.dma_start(xoutd[m * P:(m + 1) * P, :], xout[m][:])

    nc.compile()
    return nc


def kernel(dec, text_encoder, Wq1, Wk1, Wv1, Wo1, Wq2, Wk2, Wv2, Wo2, g, Wg,
           W1, W2):
    from concourse.bass_utils import run_bass_kernel_spmd
    import ml_dtypes

    if "nc" not in _cache:
        _cache["nc"] = _build()
    nc = _cache["nc"]

    dec = np.asarray(dec, np.float32)
    enc = np.asarray(text_encoder, np.float32)
    f = np.float32
    scale = np.float32(1.0 / np.sqrt(DH))
    wq1 = np.ascontiguousarray(np.asarray(Wq1, f) * scale)
    wq2 = np.ascontiguousarray(np.asarray(Wq2, f) * scale)
    gv = np.ascontiguousarray(np.asarray(g, f).reshape(KT, P).T)
    wg = np.ascontiguousarray(np.asarray(Wg, f).reshape(KT, P, 8).transpose(1, 0, 2)
                              .reshape(P, KT * 8))
    ident = np.eye(P, dtype=f)
    ones1 = np.ones((P, NH), dtype=f)
    W1b = np.asarray(W1).astype(ml_dtypes.bfloat16)
    W2b = np.asarray(W2).astype(ml_dtypes.bfloat16)

    in_maps = []
    for c in range(N_CORES):
        b, h = c // 2, c % 2
        dT = np.ascontiguousarray(dec[b].T)  # [D, 1024]
        # own queries first
        dTp = np.ascontiguousarray(
            np.concatenate([dT[:, h * TOK:(h + 1) * TOK],
                            dT[:, (1 - h) * TOK:(2 - h) * TOK]], axis=1))
        in_maps.append({
            "decT": dTp,
            "encT": np.ascontiguousarray(enc[b].T),
            "wq1": wq1, "wk1": np.asarray(Wk1, f), "wv1": np.asarray(Wv1, f),
            "wo1": np.asarray(Wo1, f),
            "wq2": wq2, "wk2": np.asarray(Wk2, f), "wv2": np.asarray(Wv2, f),
            "wo2": np.asarray(Wo2, f),
            "gv": gv, "wg": wg, "ident": ident, "ones1": ones1,
            "w1e": np.ascontiguousarray(W1b[c]),
            "w2e": np.ascontiguousarray(W2b[c]),
        })

    res = run_bass_kernel_spmd(nc, in_maps, list(range(N_CORES)))
    outs = res.results

    x = np.empty((4, 1024, D), np.float32)
    weights = np.empty((4, 1024, 2), np.float32)
    indices = np.empty((4, 1024, 2), np.int32)
    for c in range(N_CORES):
        b, h = c // 2, c % 2
        sl = slice(h * TOK, (h + 1) * TOK)
        x[b, sl, :] = outs[c]["xoutT"].T
        weights[b, sl, :] = outs[c]["wout"]
        indices[b, sl, :] = outs[c]["iout"]
    return weights, indices, x


# revision 24
# speedup vs baseline: 1.0198x; 1.0198x over previous
"""Trainium2 Bass kernel: transformer decoder layer (self-attn + cross-attn +
top-2-of-8 MoE), expert-parallel dense MoE across 8 NeuronCores.

Per-core SPMD program (core c; batch b=c//2, half h=c%2, expert e=c):
  - attention phases computed data-parallel: 512 query tokens per core, in a
    transposed [d, token] activation layout (keeps every matmul K-major).
  - x after cross-attn is AllGathered (bf16); each core runs its one expert's
    FFN densely over all 4096 tokens, scaled by the routing gate column, and
    a ReduceScatter sums expert contributions back to the token's owner core.
  - top-2 routing computed on the token's owner core in fp32.
Host preprocessing: transposes activations, pre-scales Wq by 1/sqrt(dh),
permutes each batch so the core's own query tokens are always columns 0:512.
"""

import sys

sys.path.insert(0, "/opt/trn_rl_repo")
import numpy as np

N_CORES = 8
D = 1024
DFF = 4096
NH = 16
DH = 64
TOK = 512
KV1 = 1024
KV2 = 512
P = 128
KT = D // P  # 8 contraction tiles of 128
EPS = 1e-6

_cache = {}


def _build():
    import concourse.mybir as mybir
    import concourse.tile as tile
    from concourse import bacc
    from concourse.alu_op_type import AluOpType

    f32 = mybir.dt.float32
    f16 = mybir.dt.float16
    f32r = mybir.dt.float32r
    bf16 = mybir.dt.bfloat16
    i32 = mybir.dt.int32
    u32 = mybir.dt.uint32
    ActF = mybir.ActivationFunctionType
    AX = mybir.AxisListType.X

    nc = bacc.Bacc("TRN2", target_bir_lowering=False, debug=False,
                   num_devices=N_CORES)

    decT = nc.dram_tensor("decT", [D, KV1], f32, kind="ExternalInput")
    encT = nc.dram_tensor("encT", [D, KV2], f32, kind="ExternalInput")
    wnames = ["wq1", "wk1", "wv1", "wo1", "wq2", "wk2", "wv2", "wo2"]
    WD = {n: nc.dram_tensor(n, [D, D], f32, kind="ExternalInput") for n in wnames}
    gdram = nc.dram_tensor("gv", [P, KT], f32, kind="ExternalInput")
    wgdram = nc.dram_tensor("wg", [P, KT * 8], f32, kind="ExternalInput")
    identd = nc.dram_tensor("ident", [P, P], f32, kind="ExternalInput")
    onesd = nc.dram_tensor("ones1", [P, NH], f32, kind="ExternalInput")
    w1d = nc.dram_tensor("w1e", [D, DFF], bf16, kind="ExternalInput")
    w2d = nc.dram_tensor("w2e", [DFF, D], bf16, kind="ExternalInput")

    xoutd = nc.dram_tensor("xoutT", [D, TOK], f32, kind="ExternalOutput")
    woutd = nc.dram_tensor("wout", [TOK, 2], f32, kind="ExternalOutput")
    ioutd = nc.dram_tensor("iout", [TOK, 2], i32, kind="ExternalOutput")

    with tile.TileContext(nc) as tc:
        with (
            tc.tile_pool(name="const", bufs=1) as constp,
            tc.tile_pool(name="psum", bufs=1, space="PSUM") as psp,
            tc.tile_pool(name="dram", bufs=1, space="DRAM") as dramp,
            tc.tile_pool(name="rows", bufs=3) as rowp,
        ):
            g_sb = constp.tile([P, KT], f32)
            nc.sync.dma_start(g_sb[:], gdram[:])
            wg_sb = constp.tile([P, KT * 8], f32)
            nc.sync.dma_start(wg_sb[:], wgdram[:])
            ident = constp.tile([P, P], f32)
            nc.sync.dma_start(ident[:], identd[:])
            ones_r = constp.tile([P, NH], f32r)
            nc.sync.dma_start(ones_r[:], onesd[:].bitcast(f32r))
            eps_row = constp.tile([1, 1], f32)
            nc.vector.memset(eps_row[:], EPS)

            # internal DRAM
            ag_in = dramp.tile([D, TOK], bf16)
            ag_out = dramp.tile([N_CORES * D, TOK], bf16, addr_space="Shared")
            a2a_in = dramp.tile([N_CORES, TOK], f32)
            a2a_out = dramp.tile([N_CORES, TOK], f32)
            rs_in = dramp.tile([N_CORES * D, TOK], f16)
            rs_out = dramp.tile([D, TOK], f16)
            x2dram = dramp.tile([D, TOK], f32)

            def ps_mm(n=TOK):
                return psp.tile([P, n], f32, tag="ps_mm", name="ps_mm", bufs=4)

            def w_stream(wt_dram, wtag, wpool, M=D, dt=f32r):
                tiles = []
                for k in range(KT):
                    t = wpool.tile([P, M], dt, tag=wtag, name=wtag)
                    nc.sync.dma_start(t[:], wt_dram[k * P:(k + 1) * P, :].bitcast(dt))
                    tiles.append(t)
                return tiles

            def proj(w_tiles, rhs_tiles, rhs_sl, evict, n_chunks=1):
                # out[m][nc_] = sum_k w_tiles[k][:,m*P:+P].T @ rhs_tiles[k][:, sl]
                for nci in range(n_chunks):
                    for m in range(KT):
                        ps = ps_mm()
                        for k in range(KT):
                            rhs = rhs_tiles[k][:] if rhs_sl is None else rhs_tiles[k][:, rhs_sl(nci)]
                            nc.tensor.matmul(
                                ps[:], w_tiles[k][:, m * P:(m + 1) * P], rhs,
                                start=(k == 0), stop=(k == KT - 1))
                        evict(nci, m, ps)

            def rmsnorm_apply(src_tiles, out_dtype, out_tag, out_pool,
                              sq_pool=None, sq_tag="sq", sq_bufs=8):
                # rmsnorm over the partition (d) axis of 8x[128, TOK] tiles
                if sq_pool is None:
                    sq_pool = out_pool
                sq = []
                for m in range(KT):
                    s = sq_pool.tile([P, TOK], f32r, tag=sq_tag, name="sq", bufs=sq_bufs)
                    nc.vector.tensor_mul(s[:], src_tiles[m][:], src_tiles[m][:])
                    sq.append(s)
                ps_ss = psp.tile([1, TOK], f32, tag="ps_small", name="ps_ss", bufs=2)
                for m in range(KT):
                    nc.tensor.matmul(ps_ss[:], ones_r[:, 0:1], sq[m][:],
                                     start=(m == 0), stop=(m == KT - 1))
                sqms = rowp.tile([1, TOK], f32, tag="sqms", name="sqms", bufs=1)
                nc.scalar.activation(sqms[:], ps_ss[:], ActF.Sqrt,
                                     scale=1.0 / D, bias=eps_row[:])
                rstd = rowp.tile([1, TOK], f32, tag="rstd", name="rstd", bufs=1)
                nc.vector.reciprocal(rstd[:], sqms[:])
                rbc = out_pool.tile([P, TOK], f32, tag="rbc", name="rbc", bufs=1)
                nc.gpsimd.partition_broadcast(rbc[:], rstd[:])
                outs = []
                for m in range(KT):
                    o = out_pool.tile([P, TOK], out_dtype, tag=out_tag, name=out_tag, bufs=8)
                    nc.vector.scalar_tensor_tensor(
                        o[:], src_tiles[m][:], g_sb[:, m:m + 1], rbc[:],
                        AluOpType.mult, AluOpType.mult)
                    outs.append(o)
                return outs

            def load_kv_tiles(kvT_dram, kv_len, apool):
                kv_sb = []
                for k in range(KT):
                    t = apool.tile([P, kv_len], f32r, tag="kvsrc",
                                   name="kvsrc", bufs=8)
                    nc.sync.dma_start(
                        t[:], kvT_dram[k * P:(k + 1) * P, :].bitcast(f32r))
                    kv_sb.append(t)
                return kv_sb

            def attention(kv_sb, kv_len, wq, wk, wv, wo,
                          res_fn, qsrc_fn, apool, wpool, out_pool,
                          out_dtype, out_tag):
                # transposed-layout attention for TOK query tokens.
                # qsrc_fn(k) -> [128, TOK] f32r rhs tile for the Q projection
                # res_fn(m) -> [128, TOK] f32-readable residual tile
                KVT = kv_len // P
                # Q^T = wq.T @ qsrc   [d, TOK]
                wq_t = w_stream(wq, "wproj", wpool)
                qt = []
                for m in range(KT):
                    ps = ps_mm()
                    for k in range(KT):
                        nc.tensor.matmul(ps[:], wq_t[k][:, m * P:(m + 1) * P],
                                         qsrc_fn(k), start=(k == 0),
                                         stop=(k == KT - 1))
                    t = apool.tile([P, TOK], f32r, tag="qt", name="qt", bufs=8)
                    nc.vector.tensor_copy(t[:], ps[:])
                    qt.append(t)

                # V natural [kv_tok, dv] with ones column per head (width 65)
                wv_t = w_stream(wv, "wproj", wpool)
                v_sb = []
                for t_i in range(KVT):
                    vt = apool.tile([P, NH * (DH + 1)], f32r, tag="v",
                                    name="vt", bufs=8)
                    v3 = vt[:].rearrange("p (h x) -> p h x", x=DH + 1)
                    nc.sync.dma_start(v3[:, :, DH:DH + 1],
                                      onesd[:].bitcast(f32r))
                    for nci in range(2):
                        ps = ps_mm()
                        for k in range(KT):
                            nc.tensor.matmul(
                                ps[:], kv_sb[k][:, t_i * P:(t_i + 1) * P],
                                wv_t[k][:, nci * TOK:(nci + 1) * TOK],
                                start=(k == 0), stop=(k == KT - 1))
                        nc.vector.tensor_copy(
                            v3[:, nci * 8:(nci + 1) * 8, 0:DH], ps[:])
                    v_sb.append(vt)

                # K^T tile i streamed; feeds heads 2i and 2i+1
                wk_t = w_stream(wk, "wproj", wpool)
                at_sb = [apool.tile([P, TOK], f32r, tag="at", name="at", bufs=8)
                         for _ in range(KT)]
                den16 = apool.tile([NH, TOK], f32, tag="den16", name="den16",
                                   bufs=2)
                for i in range(KT):
                    kt_i = apool.tile([P, kv_len], f32r, tag="kt", name="ktt",
                                      bufs=2)
                    for nci in range(kv_len // TOK):
                        ps = ps_mm()
                        for k in range(KT):
                            nc.tensor.matmul(
                                ps[:], wk_t[k][:, i * P:(i + 1) * P],
                                kv_sb[k][:, nci * TOK:(nci + 1) * TOK],
                                start=(k == 0), stop=(k == KT - 1))
                        nc.vector.tensor_copy(
                            kt_i[:, nci * TOK:(nci + 1) * TOK], ps[:])
                    for hh in (2 * i, 2 * i + 1):
                        po = DH * (hh % 2)
                        s_tiles = []
                        for t_i in range(KVT):
                            ps = ps_mm()
                            nc.tensor.matmul(
                                ps[:], kt_i[po:po + DH, t_i * P:(t_i + 1) * P],
                                qt[i][po:po + DH, :], start=True, stop=True)
                            st = apool.tile([P, TOK], f32r, tag="s", name="st",
                                            bufs=12)
                            nc.scalar.activation(st[:], ps[:], ActF.Exp)
                            s_tiles.append(st)
                        ps_av = psp.tile([DH + 1, TOK], f32, tag="ps_av",
                                         name="ps_av", bufs=2)
                        for t_i in range(KVT):
                            v3 = v_sb[t_i][:].rearrange("p (h x) -> p h x",
                                                        x=DH + 1)
                            nc.tensor.matmul(ps_av[:], v3[:, hh, :],
                                             s_tiles[t_i][:], start=(t_i == 0),
                                             stop=(t_i == KVT - 1))
                        dtmp = apool.tile([1, TOK], f32, tag="dtmp",
                                          name="dtmp", bufs=3)
                        nc.vector.tensor_copy(dtmp[:], ps_av[DH:DH + 1, :])
                        nc.sync.dma_start(den16[hh:hh + 1, :], dtmp[:])
                        nc.vector.tensor_copy(at_sb[i][po:po + DH, :],
                                              ps_av[0:DH, :])

                # batched softmax normalization: one reciprocal for all heads
                rec16 = apool.tile([NH, TOK], f32, tag="den16", name="rec16",
                                   bufs=2)
                nc.vector.reciprocal(rec16[:], den16[:])
                for hh in range(NH):
                    pi, po = hh // 2, DH * (hh % 2)
                    rtmp = apool.tile([1, TOK], f32, tag="dtmp",
                                      name="rtmp", bufs=3)
                    nc.sync.dma_start(rtmp[:], rec16[hh:hh + 1, :])
                    rb = apool.tile([P, TOK], f32, tag="avbc", name="rb",
                                    bufs=3)
                    nc.gpsimd.partition_broadcast(rb[:], rtmp[:])
                    nc.vector.tensor_mul(at_sb[pi][po:po + DH, :],
                                         at_sb[pi][po:po + DH, :],
                                         rb[po:po + DH, :])

                # O-proj + residual (pre tiles reuse the qt slots)
                wo_t = w_stream(wo, "wproj", wpool)
                pre = []
                for m in range(KT):
                    ps = ps_mm()
                    for k in range(KT):
                        nc.tensor.matmul(ps[:], wo_t[k][:, m * P:(m + 1) * P],
                                         at_sb[k][:], start=(k == 0),
                                         stop=(k == KT - 1))
                    t = apool.tile([P, TOK], f32, tag="qt", name="pre", bufs=8)
                    nc.vector.tensor_add(t[:], ps[:], res_fn(m))
                    pre.append(t)
                return pre

            # ---------------- phase A: self attention ----------------
            with (
                tc.tile_pool(name="attn", bufs=1) as apool,
                tc.tile_pool(name="wts", bufs=8) as wpool,
                tc.tile_pool(name="act1", bufs=1) as x1pool,
            ):
                kv1 = load_kv_tiles(decT, KV1, apool)
                pre1 = attention(
                    kv1, KV1, WD["wq1"], WD["wk1"], WD["wv1"], WD["wo1"],
                    res_fn=lambda m: kv1[m][:, 0:TOK].bitcast(f32),
                    qsrc_fn=lambda k: kv1[k][:, 0:TOK],
                    apool=apool, wpool=wpool, out_pool=x1pool,
                    out_dtype=f32r, out_tag="x12")
                x1 = rmsnorm_apply(pre1, f32r, "x12", x1pool,
                                   sq_pool=apool, sq_tag="s", sq_bufs=12)

                # ------------- phase B: cross attention -------------
                kv2 = load_kv_tiles(encT, KV2, apool)
                pre2 = attention(
                    kv2, KV2, WD["wq2"], WD["wk2"], WD["wv2"], WD["wo2"],
                    res_fn=lambda m: x1[m][:].bitcast(f32),
                    qsrc_fn=lambda k: x1[k][:],
                    apool=apool, wpool=wpool, out_pool=x1pool,
                    out_dtype=f32, out_tag="x12")
                x2 = rmsnorm_apply(pre2, f32, "x12", x1pool,
                                   sq_pool=apool, sq_tag="s", sq_bufs=12)

                # export x2: f32 -> private dram; bf16 -> allgather
                for m in range(KT):
                    nc.sync.dma_start(x2dram[m * P:(m + 1) * P, :], x2[m][:])
                    xb = x1pool.tile([P, TOK], bf16, tag="xb", name="xb", bufs=1)
                    nc.vector.tensor_copy(xb[:], x2[m][:])
                    nc.sync.dma_start(ag_in[m * P:(m + 1) * P, :], xb[:])
                nc.gpsimd.collective_compute(
                    "AllGather", mybir.AluOpType.bypass,
                    replica_groups=[list(range(N_CORES))],
                    ins=[ag_in[:].opt()], outs=[ag_out[:].opt()])

                # ------------- router (own 512 tokens, fp32) -------------
                wgv = wg_sb[:].rearrange("p (k e) -> p k e", e=8)
                ct_sb = x1pool.tile([8, TOK], f32, tag="ct", bufs=1)
                for t_i in range(TOK // P):
                    ps_l = psp.tile([P, 8], f32, tag="ps_small", name="ps_l", bufs=2)
                    for k in range(KT):
                        nc.tensor.matmul(
                            ps_l[:], x2[k][:, t_i * P:(t_i + 1) * P], wgv[:, k, :],
                            start=(k == 0), stop=(k == KT - 1))
                    negm = rowp.tile([P, 1], f32, tag="negm", name="negm", bufs=2)
                    nc.vector.reduce_max(negm[:], ps_l[:], axis=AX, negate=True)
                    eb = rowp.tile([P, 8], f32, tag="eb", name="eb", bufs=2)
                    nc.scalar.activation(eb[:], ps_l[:], ActF.Exp, bias=negm[:])
                    vals = rowp.tile([P, 8], f32, tag="vals", name="vals", bufs=2)
                    idxu = rowp.tile([P, 8], u32, tag="idxu", name="idxu", bufs=2)
                    nc.vector.max_with_indices(vals[:], idxu[:], eb[:])
                    s12 = rowp.tile([P, 1], f32, tag="s12", name="s12", bufs=2)
                    nc.vector.tensor_add(s12[:], vals[:, 0:1], vals[:, 1:2])
                    rr = rowp.tile([P, 1], f32, tag="rr", name="rr", bufs=2)
                    nc.vector.reciprocal(rr[:], s12[:])
                    wo_sb = rowp.tile([P, 2], f32, tag="wo_sb", name="wo_sb", bufs=2)
                    nc.vector.tensor_scalar_mul(wo_sb[:], vals[:, 0:2], rr[:])
                    nc.sync.dma_start(woutd[t_i * P:(t_i + 1) * P, :], wo_sb[:])
                    io_sb = rowp.tile([P, 2], i32, tag="io_sb", name="io_sb", bufs=2)
                    nc.vector.tensor_copy(io_sb[:], idxu[:, 0:2])
                    nc.sync.dma_start(ioutd[t_i * P:(t_i + 1) * P, :], io_sb[:])
                    mask = rowp.tile([P, 8], f32, tag="mask", name="mask", bufs=2)
                    nc.vector.tensor_scalar(mask[:], eb[:], vals[:, 1:2], None,
                                            AluOpType.is_ge)
                    comb = rowp.tile([P, 8], f32, tag="comb", name="comb", bufs=2)
                    nc.vector.scalar_tensor_tensor(
                        comb[:], eb[:], rr[:], mask[:],
                        AluOpType.mult, AluOpType.mult)
                    ps_t = psp.tile([8, P], f32, tag="ps_small", name="ps_t", bufs=2)
                    nc.tensor.transpose(ps_t[:], comb[:], ident[:])
                    nc.vector.tensor_copy(ct_sb[:, t_i * P:(t_i + 1) * P], ps_t[:])
                nc.sync.dma_start(a2a_in[:], ct_sb[:])
                nc.gpsimd.collective_compute(
                    "AllToAll", mybir.AluOpType.bypass,
                    replica_groups=[list(range(N_CORES))],
                    ins=[a2a_in[:].opt()], outs=[a2a_out[:].opt()])

            # ---------------- phase C: dense expert FFN ----------------
            with (
                tc.tile_pool(name="moew", bufs=1) as mwp,
                tc.tile_pool(name="moea", bufs=1) as map_,
                tc.tile_pool(name="xg", bufs=16) as xgp,
                tc.tile_pool(name="ysb", bufs=3) as yp,
            ):
                w1_sb = []
                for k in range(KT):
                    t = mwp.tile([P, DFF], bf16, tag="w1", name="w1t", bufs=8)
                    nc.sync.dma_start(t[:], w1d[k * P:(k + 1) * P, :])
                    w1_sb.append(t)
                w2_sb = []
                for k in range(DFF // P):
                    t = mwp.tile([P, D], bf16, tag="w2", name="w2t", bufs=32)
                    nc.sync.dma_start(t[:], w2d[k * P:(k + 1) * P, :])
                    w2_sb.append(t)

                for r in range(N_CORES):
                    xg = []
                    for k in range(KT):
                        t = xgp.tile([P, TOK], bf16, tag="xg", name="xgt")
                        nc.sync.dma_start(
                            t[:], ag_out[r * D + k * P: r * D + (k + 1) * P, :])
                        xg.append(t)
                    cvec = rowp.tile([1, TOK], f32, tag="cvec", name="cvec", bufs=1)
                    nc.sync.dma_start(cvec[:], a2a_out[r:r + 1, :])
                    cbc = yp.tile([P, TOK], f32, tag="cbc", name="cbc", bufs=2)
                    nc.gpsimd.partition_broadcast(cbc[:], cvec[:])

                    h_sb = []
                    for mf in range(DFF // P):
                        ps = ps_mm()
                        for k in range(KT):
                            nc.tensor.matmul(
                                ps[:], w1_sb[k][:, mf * P:(mf + 1) * P], xg[k][:],
                                start=(k == 0), stop=(k == KT - 1))
                        ht = map_.tile([P, TOK], bf16, tag="h", name="ht", bufs=32)
                        nc.scalar.activation(ht[:], ps[:], ActF.Relu)
                        h_sb.append(ht)

                    for md in range(KT):
                        ps = ps_mm()
                        for kk in range(DFF // P):
                            nc.tensor.matmul(
                                ps[:], w2_sb[kk][:, md * P:(md + 1) * P], h_sb[kk][:],
                                start=(kk == 0), stop=(kk == DFF // P - 1))
                        yt = yp.tile([P, TOK], f16, tag="y", name="yt")
                        nc.vector.tensor_mul(yt[:], ps[:], cbc[:])
                        nc.sync.dma_start(
                            rs_in[r * D + md * P: r * D + (md + 1) * P, :], yt[:])

                nc.gpsimd.collective_compute(
                    "ReduceScatter", mybir.AluOpType.add,
                    replica_groups=[list(range(N_CORES))],
                    ins=[rs_in[:].opt()], outs=[rs_out[:].opt()])

            # ---------------- phase D: final residual + norm ----------------
            with tc.tile_pool(name="fin", bufs=1) as fp:
                pre = []
                for m in range(KT):
                    yt = fp.tile([P, TOK], f16, tag="yl", name="ylt", bufs=3)
                    nc.sync.dma_start(yt[:], rs_out[m * P:(m + 1) * P, :])
                    xt = fp.tile([P, TOK], f32, tag="xl", name="xlt", bufs=3)
                    nc.sync.dma_start(xt[:], x2dram[m * P:(m + 1) * P, :])
                    t = fp.tile([P, TOK], f32, tag="fpre", name="fpre", bufs=8)
                    nc.vector.tensor_add(t[:], yt[:], xt[:])
                    pre.append(t)
                xout = rmsnorm_apply(pre, f32, "xout", fp)
                for m in range(KT):
                    nc.sync.dma_start(xoutd[m * P:(m + 1) * P, :], xout[m][:])

    nc.compile()
    return nc


def kernel(dec, text_encoder, Wq1, Wk1, Wv1, Wo1, Wq2, Wk2, Wv2, Wo2, g, Wg,
           W1, W2):
    from concourse.bass_utils import run_bass_kernel_spmd
    import ml_dtypes

    if "nc" not in _cache:
        _cache["nc"] = _build()
    nc = _cache["nc"]

    dec = np.asarray(dec, np.float32)
    enc = np.asarray(text_encoder, np.float32)
    f = np.float32
    scale = np.float32(1.0 / np.sqrt(DH))
    wq1 = np.ascontiguousarray(np.asarray(Wq1, f) * scale)
    wq2 = np.ascontiguousarray(np.asarray(Wq2, f) * scale)
    gv = np.ascontiguousarray(np.asarray(g, f).reshape(KT, P).T)
    wg = np.ascontiguousarray(np.asarray(Wg, f).reshape(KT, P, 8).transpose(1, 0, 2)
                              .reshape(P, KT * 8))
    ident = np.eye(P, dtype=f)
    ones1 = np.ones((P, NH), dtype=f)
    W1b = np.asarray(W1).astype(ml_dtypes.bfloat16)
    W2b = np.asarray(W2).astype(ml_dtypes.bfloat16)

    in_maps = []
    for c in range(N_CORES):
        b, h = c // 2, c % 2
        dT = np.ascontiguousarray(dec[b].T)  # [D, 1024]
        # own queries first
        dTp = np.ascontiguousarray(
            np.concatenate([dT[:, h * TOK:(h + 1) * TOK],
                            dT[:, (1 - h) * TOK:(2 - h) * TOK]], axis=1))
        in_maps.append({
            "decT": dTp,
            "encT": np.ascontiguousarray(enc[b].T),
            "wq1": wq1, "wk1": np.asarray(Wk1, f), "wv1": np.asarray(Wv1, f),
            "wo1": np.asarray(Wo1, f),
            "wq2": wq2, "wk2": np.asarray(Wk2, f), "wv2": np.asarray(Wv2, f),
            "wo2": np.asarray(Wo2, f),
            "gv": gv, "wg": wg, "ident": ident, "ones1": ones1,
            "w1e": np.ascontiguousarray(W1b[c]),
            "w2e": np.ascontiguousarray(W2b[c]),
        })

    res = run_bass_kernel_spmd(nc, in_maps, list(range(N_CORES)))
    outs = res.results

    x = np.empty((4, 1024, D), np.float32)
    weights = np.empty((4, 1024, 2), np.float32)
    indices = np.empty((4, 1024, 2), np.int32)
    for c in range(N_CORES):
        b, h = c // 2, c % 2
        sl = slice(h * TOK, (h + 1) * TOK)
        x[b, sl, :] = outs[c]["xoutT"].T
        weights[b, sl, :] = outs[c]["wout"]
        indices[b, sl, :] = outs[c]["iout"]
    return weights, indices, x
